# revision 1
# baseline (speedup 1.0000x reference)
"""Multi-head attention layer on 8 Trainium2 NeuronCores.

Reference (per batch n):
    Q = x@Wq + bq; K = x@Wk + bk; V = x@Wv + bv       (16 heads, Dh=64)
    out = softmax(Q K^T / sqrt(Dh)) V  -> concat heads -> @Wo + bo

Sharding: 2 head-groups (tensor parallel) x 4 batches (data parallel) = 8
cores. Core c handles batch c%4 and heads [8*(c//4), 8*(c//4)+8). Each core
computes a partial output projection with its Wo row-block; the host sums
the two head-group partials per batch (the only cross-core reduction).

Per-core kernel (bf16 matmul inputs, fp32 PSUM accumulation):
  - projections from a host-pre-transposed x^T: K^T,Q^T in [d_head(part),
    seq] layout padded to a full 128-row contraction (so every matmul uses
    the whole PE array and keeps its clock gate warm), V in [seq(part),
    d_head] layout with an appended ones column (softmax denominators fall
    out of the PV matmul for free).
  - attention per head: S^T = K Q^T on TensorE; exp on ScalarE over
    [128,1024] PSUM tiles (scale=1/8 folded in; no max subtraction --
    scores are O(1) by construction); O^T += V_aug^T exp(S^T) accumulated
    over 16 seq-tiles; normalization multiplies by a broadcast reciprocal
    of the denominator row (K=1 ones matmul broadcast, float32r).
  - partial out-projection with Wo rows (+bo on group 0 only).
Scheduling: the exp chain on ScalarE is the critical resource. Projection
and out-projection work is dribbled one matmul at a time into the PE slack
between attention matmuls (deadline-ordered filler queue), normalization
tails are deferred into the next head's loop, and PSUM accumulator banks
are released immediately at each head boundary.

Self-contained: hardcodes shapes for x:[4,2048,1024], d_model=1024,
16 heads; a no-bias graph variant is compiled when all biases are zero.
"""

import sys
import types

import numpy as np

import concourse.mybir as mybir
import concourse.tile as tile
from concourse import bacc
from concourse.bass_utils import run_bass_kernel_spmd

f32 = mybir.dt.float32
f32r = mybir.dt.float32r
bf16 = mybir.dt.bfloat16
AF = mybir.ActivationFunctionType

N_CORES = 8
P = 128

# ---------------------------------------------------------------------------


def build_nc(L=2048, D=1024, HPC=8, Dh=64, WB=True):
    """Build the per-core Bass graph (SPMD: same graph, per-core shards)."""
    KO = D // P          # k-tiles over d_model
    DQ = HPC * Dh        # local projected dim
    DKC = DQ // P        # 128-row chunks of DQ
    NSC = L // 512       # 512-wide seq chunks
    ST = L // P          # 128-row seq tiles
    MC = L // 512        # 512-wide m chunks
    WKO = DQ // P        # k-tiles for out-proj contraction
    EC = D // 512        # 512-wide out chunks
    MS = L // P          # 128-row out row-tiles
    assert MC % 2 == 0 and HPC % 2 == 0

    nc = bacc.Bacc("TRN2", target_bir_lowering=False, debug=False,
                   num_devices=N_CORES)

    xT_d = nc.dram_tensor("xT", [D, L], bf16, kind="ExternalInput")
    Wq_d = nc.dram_tensor("Wq", [D, DQ], bf16, kind="ExternalInput")
    Wk_d = nc.dram_tensor("Wk", [D, DQ], bf16, kind="ExternalInput")
    Wv_d = nc.dram_tensor("Wv", [D, DQ], bf16, kind="ExternalInput")
    Wo_d = nc.dram_tensor("Wo", [DQ, D], bf16, kind="ExternalInput")
    bq_d = nc.dram_tensor("bq", [DQ], bf16, kind="ExternalInput")
    bk_d = nc.dram_tensor("bk", [DQ], bf16, kind="ExternalInput")
    bv_d = nc.dram_tensor("bv", [DQ], bf16, kind="ExternalInput")
    bo_d = nc.dram_tensor("bo", [D], bf16, kind="ExternalInput")
    out_d = nc.dram_tensor("out", [L, D], f32, kind="ExternalOutput")

    xT_v = xT_d.ap().rearrange("(ko p) s -> p ko s", p=P)
    Wq_v = Wq_d.ap().rearrange("(ko p) d -> p ko d", p=P)
    Wk_v = Wk_d.ap().rearrange("(ko p) d -> p ko d", p=P)
    Wv_v = Wv_d.ap().rearrange("(ko p) d -> p ko d", p=P)
    Wo_v = Wo_d.ap().rearrange("(ko p) e -> p ko e", p=P)
    out_v = out_d.ap().rearrange("(ms p) e -> p ms e", p=P)

    with tile.TileContext(nc) as tc:
        with (
            tc.tile_pool(name="pp", bufs=1) as pp,
            tc.tile_pool(name="wp", bufs=1) as wp,
            tc.tile_pool(name="sp", bufs=1) as sp,
            tc.tile_pool(name="ps", bufs=1, space="PSUM") as ps,
        ):
            # ---- persistent tiles ----
            KT = pp.tile([P, HPC, L], bf16, name="KT")
            QT = pp.tile([P, HPC, L], bf16, name="QT")
            VA = pp.tile([P, ST, HPC, Dh + 1], bf16, name="VA")
            OT = pp.tile([P, WKO, L], bf16, name="OT")
            ones_f = pp.tile([P, P], f32, name="ones_f")
            ones_r = pp.tile([P, P], f32r, name="ones_r")
            ones_b = pp.tile([1, 512], bf16, name="ones_b")
            nc.vector.memset(ones_f[:], 1.0)
            nc.vector.tensor_copy(ones_r[:], ones_f[:])
            nc.vector.memset(ones_b[:], 1.0)
            nc.vector.tensor_copy(VA[:, :, :, Dh:Dh + 1],
                                  ones_f[:, 0:1].to_broadcast((P, ST, HPC, 1)))
            bqs = pp.tile([1, DQ], bf16, name="bqs")
            bks = pp.tile([1, DQ], bf16, name="bks")
            bvs = pp.tile([1, DQ], bf16, name="bvs")
            bos = pp.tile([1, D], bf16, name="bos")
            nc.sync.dma_start(bqs[:], bq_d.ap()[None, :])
            nc.sync.dma_start(bks[:], bk_d.ap()[None, :])
            nc.gpsimd.memset(KT[64:128, :, :], 0.0)
            nc.gpsimd.memset(QT[64:128, :, :], 0.0)
            nc.sync.dma_start(bvs[:], bv_d.ap()[None, :])
            nc.sync.dma_start(bos[:], bo_d.ap()[None, :])

            # Wv resident for phase A
            Wv_sb = wp.tile([P, KO, DQ], bf16, name="Wv_sb")
            nc.sync.dma_start(Wv_sb[:], Wv_v)

            # ---- phase A1: V + K^T projections, emitted per seq-chunk ----
            xts_tiles = []

            def issue_xts_dma(sc):
                xts = sp.tile([P, KO, 512], bf16, tag="xts", bufs=NSC,
                              name=f"xts{sc}")
                nc.sync.dma_start(xts[:], xT_v[:, :, sc * 512:(sc + 1) * 512])
                xts_tiles.append(xts)

            def emit_v(sc):
                xts = xts_tiles[sc]
                for ssub in range(4):
                    st = sc * 4 + ssub
                    pv = ps.tile([P, 512], f32, tag="b512", bufs=3,
                                 name=f"pv{st}")
                    for ko in range(KO):
                        nc.tensor.matmul(
                            pv[:, 0:DQ],
                            lhsT=xts[:, ko, ssub * P:(ssub + 1) * P],
                            rhs=Wv_sb[:, ko, :],
                            start=(ko == 0), stop=(not WB and ko == KO - 1))
                    if WB:
                        nc.tensor.matmul(pv[:, 0:DQ],
                                         lhsT=ones_b[0:1, 0:P],
                                         rhs=bvs[0:1, :],
                                         start=False, stop=True)
                    nc.vector.tensor_copy(
                        VA[:, st, :, 0:Dh],
                        pv[:, 0:DQ].rearrange("p (h d) -> p h d", d=Dh))

            def kt_steps(dkc, sc):
                xts = xts_tiles[sc]
                wt = sp.tile([P, KO, P], bf16, tag="wk", bufs=2,
                             name=f"wk{sc}_{dkc}")
                nc.sync.dma_start(wt[:], Wk_v[:, :, dkc * P:(dkc + 1) * P])
                pt = ps.tile([P, 512], f32, tag="b512", bufs=3,
                             name=f"pk{sc}_{dkc}")
                for ko in range(KO):
                    nc.tensor.matmul(pt[:], lhsT=wt[:, ko, :],
                                     rhs=xts[:, ko, :],
                                     start=(ko == 0),
                                     stop=(not WB and ko == KO - 1))
                    yield
                if WB:
                    nc.tensor.matmul(
                        pt[:], lhsT=bks[0:1, dkc * P:(dkc + 1) * P],
                        rhs=ones_b[0:1, 0:512], start=False, stop=True)
                ssl = slice(sc * 512, (sc + 1) * 512)
                nc.vector.tensor_copy(KT[0:64, 2 * dkc, ssl], pt[0:64, :])
                nc.vector.tensor_copy(KT[0:64, 2 * dkc + 1, ssl],
                                      pt[64:128, :])
                yield

            def emit_kt(dkc, sc):
                for _ in kt_steps(dkc, sc):
                    pass

            def emit_qt(dkc, sc):
                """Q^T projection for one (dq-chunk, seq-chunk)."""
                wt = sp.tile([P, KO, P], bf16, tag="wq", bufs=2,
                             name=f"wq{sc}_{dkc}")
                nc.sync.dma_start(wt[:], Wq_v[:, :, dkc * P:(dkc + 1) * P])
                pt = ps.tile([P, 512], f32, tag="b512", bufs=3,
                             name=f"pq{sc}_{dkc}")
                for ko in range(KO):
                    nc.tensor.matmul(pt[:], lhsT=wt[:, ko, :],
                                     rhs=xts_tiles[sc][:, ko, :],
                                     start=(ko == 0),
                                     stop=(not WB and ko == KO - 1))
                if WB:
                    nc.tensor.matmul(
                        pt[:], lhsT=bqs[0:1, dkc * P:(dkc + 1) * P],
                        rhs=ones_b[0:1, 0:512], start=False, stop=True)
                ssl = slice(sc * 512, (sc + 1) * 512)
                nc.scalar.copy(QT[0:64, 2 * dkc, ssl], pt[0:64, :])
                nc.vector.tensor_copy(QT[0:64, 2 * dkc + 1, ssl],
                                      pt[64:128, :])

            def emit_outproj(ms, Wo_sb):
                """Partial out-projection for one 128-row tile."""
                for ec in range(EC):
                    pt = ps.tile([P, 512], f32, tag="b512", bufs=3,
                                 name=f"po{ms}_{ec}")
                    for ko in range(WKO):
                        nc.tensor.matmul(
                            pt[:], lhsT=OT[:, ko, ms * P:(ms + 1) * P],
                            rhs=Wo_sb[:, ko, ec * 512:(ec + 1) * 512],
                            start=(ko == 0),
                            stop=(not WB and ko == WKO - 1))
                    if WB:
                        nc.tensor.matmul(pt[:], lhsT=ones_b[0:1, 0:P],
                                         rhs=bos[0:1,
                                                 ec * 512:(ec + 1) * 512],
                                         start=False, stop=True)
                    os_ = sp.tile([P, 512], f32, tag="os", bufs=3,
                                  name=f"os{ms}_{ec}")
                    nc.vector.tensor_copy(os_[:], pt[:])
                    nc.sync.dma_start(out_v[:, ms, ec * 512:(ec + 1) * 512],
                                      os_[:])

            def qt_steps(dkc, sc):
                """emit_qt broken into single-matmul steps (PE filler)."""
                wt = sp.tile([P, KO, P], bf16, tag="wq", bufs=2,
                             name=f"wqf{sc}_{dkc}")
                nc.sync.dma_start(wt[:], Wq_v[:, :, dkc * P:(dkc + 1) * P])
                pt = ps.tile([P, 512], f32, tag="b512", bufs=3,
                             name=f"pqf{sc}_{dkc}")
                for ko in range(KO):
                    nc.tensor.matmul(pt[:], lhsT=wt[:, ko, :],
                                     rhs=xts_tiles[sc][:, ko, :],
                                     start=(ko == 0),
                                     stop=(not WB and ko == KO - 1))
                    yield
                if WB:
                    nc.tensor.matmul(
                        pt[:], lhsT=bqs[0:1, dkc * P:(dkc + 1) * P],
                        rhs=ones_b[0:1, 0:512], start=False, stop=True)
                ssl = slice(sc * 512, (sc + 1) * 512)
                nc.vector.tensor_copy(QT[0:64, 2 * dkc, ssl], pt[0:64, :])
                nc.vector.tensor_copy(QT[0:64, 2 * dkc + 1, ssl],
                                      pt[64:128, :])
                yield

            def outproj_steps(ms, Wo_sb):
                """emit_outproj broken into single-matmul steps."""
                for ec in range(EC):
                    pt = ps.tile([P, 512], f32, tag="b512", bufs=3,
                                 name=f"pof{ms}_{ec}")
                    for ko in range(WKO):
                        nc.tensor.matmul(
                            pt[:], lhsT=OT[:, ko, ms * P:(ms + 1) * P],
                            rhs=Wo_sb[:, ko, ec * 512:(ec + 1) * 512],
                            start=(ko == 0),
                            stop=(not WB and ko == WKO - 1))
                        yield
                    if WB:
                        nc.tensor.matmul(pt[:], lhsT=ones_b[0:1, 0:P],
                                         rhs=bos[0:1,
                                                 ec * 512:(ec + 1) * 512],
                                         start=False, stop=True)
                    os_ = sp.tile([P, 512], f32, tag="os", bufs=3,
                                  name=f"osf{ms}_{ec}")
                    nc.vector.tensor_copy(os_[:], pt[:])
                    nc.sync.dma_start(out_v[:, ms, ec * 512:(ec + 1) * 512],
                                      os_[:])
                    yield

            import itertools

            def emit_norm_tail(item):
                """Broadcast-reciprocal matmul + normalize, deferred so the
                PE never waits on the DVE normalization chain."""
                dnr, ot, h_, mc_ = item
                bp = ps.tile([Dh, 512], f32, tag="bp", bufs=1,
                             name=f"bp{h_}_{mc_}")
                nc.tensor.matmul(bp[:], lhsT=ones_r[0:1, 0:Dh],
                                 rhs=dnr[0:1, :], start=True, stop=True)
                half = Dh * (h_ % 2)
                dkc = h_ // 2
                nc.vector.tensor_tensor(
                    OT[half:half + Dh, dkc, mc_ * 512:(mc_ + 1) * 512],
                    ot[:], bp[:], mybir.AluOpType.mult)

            pending = []

            def drain_overdue(fq, h):
                """Fully emit any filler chunk that head h depends on."""
                while fq and fq[0][1] <= h:
                    _, _, gen = fq.pop(0)
                    for _ in gen:
                        pass

            def filler_step(fq, h):
                """Advance the front filler chunk by one matmul if allowed."""
                if fq and fq[0][0] <= h:
                    try:
                        next(fq[0][2])
                    except StopIteration:
                        fq.pop(0)
                        filler_step(fq, h)

            def emit_b_head(mcg, h, fq):
                drain_overdue(fq, h)
                ops = []
                for mci in range(2):
                    op = ps.tile([P, 512], f32, tag="b512", bufs=3,
                                 name=f"op{h}_{mcg}_{mci}")
                    ops.append(op)
                def emit_s(st):
                    spt = ps.tile([P, 1024], f32, tag="b1024", bufs=2,
                                  name=f"sp{h}_{mcg}_{st}")
                    for mci in range(2):
                        mc = mcg * 2 + mci
                        nc.tensor.matmul(
                            spt[:, mci * 512:(mci + 1) * 512],
                            lhsT=KT[:, h, st * P:(st + 1) * P],
                            rhs=QT[:, h, mc * 512:(mc + 1) * 512],
                            start=True, stop=True)
                    return spt

                # software-pipelined: S^T for st+1 is emitted before PV(st)
                # so the in-order PE queue never parks a waiting PV in front
                # of the next chunk of exp input
                spts = emit_s(0)
                for st in range(ST):
                    es = sp.tile([P, 1024], bf16, tag="es", bufs=5,
                                 name=f"es{h}_{mcg}_{st}")
                    nc.scalar.activation(es[:], spts[:], AF.Exp, scale=0.125)
                    if st + 1 < ST:
                        spts = emit_s(st + 1)
                    for mci in range(2):
                        nc.tensor.matmul(
                            ops[mci][0:Dh + 1, :],
                            lhsT=VA[:, st, h, :],
                            rhs=es[:, mci * 512:(mci + 1) * 512],
                            start=(st == 0), stop=(st == ST - 1))
                    if st in (ST // 3, (2 * ST) // 3) and pending:
                        emit_norm_tail(pending.pop(0))
                    else:
                        filler_step(fq, h)
                        if mcg == 0 and (h >= 2 or st >= 10):
                            filler_step(fq, h)
                # head end: free accumulator banks fast, prep reciprocals
                dns, ots = [], []
                for mci in range(2):
                    mc = mcg * 2 + mci
                    op = ops[mci]
                    dn = sp.tile([1, 512], f32, tag="dn", bufs=6,
                                 name=f"dn{h}_{mc}")
                    nc.vector.tensor_copy(dn[:], op[Dh:Dh + 1, :])
                    ot = sp.tile([Dh, 512], f32, tag="ott", bufs=4,
                                 name=f"ot{h}_{mc}")
                    nc.vector.tensor_copy(ot[:], op[0:Dh, :])
                    dns.append(dn)
                    ots.append(ot)
                for mci in range(2):
                    dn, ot = dns[mci], ots[mci]
                    nc.vector.reciprocal_approx_fast(dn[:], dn[:])
                    dnr = sp.tile([1, 512], f32r, tag="dnr", bufs=6,
                                  name=f"dnr{h}_{mcg * 2 + mci}")
                    nc.vector.tensor_copy(dnr[:], dn[:])
                    pending.append((dnr, ot, h, mcg * 2 + mci))

            # prologue: all V projections (every head consumes all of V),
            # K^T for the first two head-pairs, Q^T for head-pair 0; the rest
            # of K^T/Q^T dribbles through the attention loop as PE filler
            for sc in range(NSC):
                issue_xts_dma(sc)
            for sc in range(NSC):
                emit_v(sc)
            if MC // 2 == 1:
                for dkc in range(DKC):
                    for sc in range(NSC):
                        emit_kt(dkc, sc)
                for dkc in range(DKC):
                    for scq in range(NSC):
                        emit_qt(dkc, scq)
            else:
                for sc in range(NSC):
                    emit_kt(0, sc)
                emit_qt(0, 0)
                emit_qt(0, 1)
            Wo_sb = wp.tile([P, WKO, D], bf16, name="Wo_sb")
            nc.sync.dma_start(Wo_sb[:], Wo_v)

            for mcg in range(MC // 2):
                fq = []
                if mcg == 0 and MC // 2 > 1:
                    fq += [(0, 2, kt_steps(1, s)) for s in range(NSC)]
                    fq += [(0, 2, qt_steps(1, 0)), (0, 2, qt_steps(1, 1))]
                    for dkc in range(2, DKC):
                        fq += [(0, 2 * dkc, kt_steps(dkc, s))
                               for s in range(NSC)]
                        fq += [(0, 2 * dkc, qt_steps(dkc, 0)),
                               (0, 2 * dkc, qt_steps(dkc, 1))]
                    fq += [(0, HPC, qt_steps(0, 2)), (0, HPC, qt_steps(0, 3))]
                elif mcg > 0:
                    for dkc in range(1, DKC):
                        fq += [(0, 2 * dkc, qt_steps(dkc, 2)),
                               (0, 2 * dkc, qt_steps(dkc, 3))]
                    fq += [(1, HPC, outproj_steps(ms, Wo_sb))
                           for ms in range((mcg - 1) * HPC, mcg * HPC)]
                for h in range(HPC):
                    emit_b_head(mcg, h, fq)
                # m-half done: drain filler; normalizations of the last head
                # stay deferred into the next m-half (flushed after the last)
                drain_overdue(fq, HPC)

            # remaining out-projection rows (last m-half): each row-tile only
            # needs the normalization of its own m-chunk, so flush the
            # deferred tails just in time to keep the PE stream dense
            base = HPC * (MC // 2 - 1)
            for ms in range(base, MS):
                due = [it for it in pending if it[3] <= (ms * P) // 512]
                for it in due:
                    pending.remove(it)
                    emit_norm_tail(it)
                emit_outproj(ms, Wo_sb)
            while pending:
                emit_norm_tail(pending.pop(0))

    nc.compile()
    return nc


# ---------------------------------------------------------------------------

_NC_CACHE = {}


def _get_nc(with_biases=True):
    key = ("nc", with_biases)
    if key not in _NC_CACHE:
        _NC_CACHE[key] = build_nc(WB=with_biases)
    return _NC_CACHE[key]


def _install_ntff_hook():
    """Provide antenv.axon_hooks (absent in this image) so trace=True can
    capture NTFF profiles for timing."""
    if "antenv.axon_hooks" in sys.modules:
        return
    mod = types.ModuleType("antenv.axon_hooks")
    holder = [None]
    mod.set_axon_ntff_profile_hook = lambda hk: holder.__setitem__(0, hk)
    mod.get_axon_ntff_profile_hook = lambda: holder[0]
    sys.modules["antenv.axon_hooks"] = mod
    import antenv

    antenv.axon_hooks = mod
    try:
        from trn_agent_boot.trn_boot import _ntff_profile_via_ctypes

        mod.set_axon_ntff_profile_hook(
            _ntff_profile_via_ctypes("/opt/axon/libaxon_pjrt.so"))
    except Exception:
        pass


def _make_in_maps(x, Wq, bq, Wk, bk, Wv, bv, Wo, bo):
    import ml_dtypes

    NB, L, D = x.shape          # 4, 2048, 1024
    DQ = D // 2                 # head-group width (8 heads x 64)
    in_maps = []
    for c in range(N_CORES):
        n, g = c % 4, c // 4
        sl = slice(g * DQ, (g + 1) * DQ)
        in_maps.append({
            "xT": np.ascontiguousarray(x[n].T).astype(ml_dtypes.bfloat16),
            "Wq": np.ascontiguousarray(Wq[:, sl]).astype(ml_dtypes.bfloat16),
            "Wk": np.ascontiguousarray(Wk[:, sl]).astype(ml_dtypes.bfloat16),
            "Wv": np.ascontiguousarray(Wv[:, sl]).astype(ml_dtypes.bfloat16),
            "Wo": np.ascontiguousarray(Wo[sl, :]).astype(ml_dtypes.bfloat16),
            "bq": np.ascontiguousarray(bq[sl]).astype(ml_dtypes.bfloat16),
            "bk": np.ascontiguousarray(bk[sl]).astype(ml_dtypes.bfloat16),
            "bv": np.ascontiguousarray(bv[sl]).astype(ml_dtypes.bfloat16),
            "bo": (bo if g == 0 else np.zeros_like(bo)).astype(
                ml_dtypes.bfloat16),
        })
    return in_maps


def run_sharded(inputs, trace=False):
    """Run the SPMD kernel on the full inputs. Returns (output, exec_time_ns)."""
    wb = any(
        np.asarray(inputs[k]).any() for k in ("bq", "bk", "bv", "bo"))
    nc = _get_nc(with_biases=bool(wb))
    if trace:
        _install_ntff_hook()
    in_maps = _make_in_maps(**inputs)
    res = run_bass_kernel_spmd(nc, in_maps, list(range(N_CORES)), trace=trace)
    outs = [res.results[c]["out"] for c in range(N_CORES)]
    full = np.stack([outs[n] + outs[n + 4] for n in range(4)], axis=0)
    return full.astype(np.float32), res.exec_time_ns


def kernel(**inputs):
    out, _ = run_sharded(inputs, trace=False)
    return out

